# revision 84
# baseline (speedup 1.0000x reference)
"""GAT self-attention Trainium2 kernel (v3).

Full inputs -> shard graphs over 8 NeuronCores -> full output.

Math (per graph n, reference reformulated):
  g_i = sigmoid(relu(q @ W1_i) @ W2_i)            [2d]
  u_i^L = W_i @ (g_i[:d] * a_i[:d])               [k]   (left projector)
  u_i^R = W_i @ (g_i[d:] * a_i[d:])               [k]   (right projector)
  left_i = X @ u_i^L ; right_i = X @ u_i^R        [E]
  score[i,j] = prelu(left_t[i] + right_t[j]), t = adj[i,j]; -BIG if adj==0
  Ex = exp(score); rs = rowsum(Ex); coefs = Ex / rs[:,None]
  h = X @ W_2 ; out = coefs^T @ h

Host pre-work (free wrt device time): x pre-transposed to [K, E] fp16,
W_type pre-transposed, masks (adj==2 / ==3 / ==0) precomputed as u8,
all weights fp16.  Gate matmuls run weight-stationary so their outputs
land directly in d-major layout (no transposes).  The main loop is
software-pipelined: L/R score rows and h for later graphs, the
select/prelu chain for graph n, softmax+normalize for graph n-1, and
the output matmuls for graph n-1 all overlap; candidate planes use two
3-deep PSUM rotations so selects never serialize on banks.
"""
import numpy as np
from contextlib import ExitStack

import concourse.bass as bass
import concourse.tile as tile
from concourse import mybir, bacc
F32 = mybir.dt.float32
BF16 = mybir.dt.float16   # fp16: same engine throughput as bf16, 4x mantissa
U8 = mybir.dt.uint8
AF = mybir.ActivationFunctionType
OP = mybir.AluOpType

N_CORES = 8
N, E, K, D = 64, 512, 512, 512   # graphs, entities, in_dim, out_dim
NG = N // N_CORES                # graphs per core
NT = 3                           # edge types
P = 128
EC = E // P                      # 4 partition chunks of E
KC = K // P
D2 = 2 * D                       # gate dim (1024)
NEG_BIG = -200.0
LRELU_SLOPE = 0.2
USE_PRELU = True   # ACT Prelu not implemented in CoreSim; set False for sim

BF = mybir.dt.np(BF16)           # numpy fp16


def build(nc, reps=1):
    xT = nc.dram_tensor("xT", [NG, K, E], BF16, kind="ExternalInput").ap()
    msk = nc.dram_tensor("msk", [NG, NT, E, E], U8, kind="ExternalInput").ap()
    qT = nc.dram_tensor("qT", [K, NG], BF16, kind="ExternalInput").ap()
    W1 = nc.dram_tensor("W1", [NT, K, D2], BF16, kind="ExternalInput").ap()
    W2q = nc.dram_tensor("W2q", [NT, D2, D2], BF16, kind="ExternalInput").ap()
    WT = nc.dram_tensor("WT", [NT, D, K], BF16, kind="ExternalInput").ap()
    W2 = nc.dram_tensor("W2", [K, D], BF16, kind="ExternalInput").ap()
    arep = nc.dram_tensor("arep", [NT, D2], BF16, kind="ExternalInput").ap()
    out = nc.dram_tensor("out", [NG, E, D], BF16, kind="ExternalOutput").ap()
    nc._gat_io = (xT, msk, qT, W1, W2q, WT, W2, arep, out)
    _build_once(nc, reps)


def _build_once(nc, reps=1):
    xT, msk, qT, W1, W2q, WT, W2, arep, out = nc._gat_io
    with tile.TileContext(nc) as tc, ExitStack() as ctx:
        # ---------------- persistent tiles ----------------
        pers = ctx.enter_context(tc.tile_pool(name="pers", bufs=1))
        ident_bf = pers.tile([P, P], BF16)
        make_identity(nc, ident_bf[:])

        negpl = pers.tile([P, E], F32)
        nc.vector.memset(negpl[:], NEG_BIG)
        expbias = pers.tile([P, 1], F32)
        nc.vector.memset(expbias[:], -2.0)
        W2sb = pers.tile([P, KC, D], BF16)
        nc.sync.dma_start(W2sb[:], W2.rearrange("(c p) d -> p c d", p=P))
        # U_all[k%128, kc, c, n], c = t + 3s: (L1,L2,L3,R1,R2,R3)
        U_all = pers.tile([P, KC, 2 * NT, NG], BF16)
        # ping-pong score operand tiles (PE lhsT/rhs need base partition 0):
        # lhs_all[0,t,:] = L_t, row 1 = ones; rhs_all[0] = ones, [1,t,:] = R_t
        lhs_tiles = [None, None]
        rhs_tiles = [None, None]
        for par in range(2):
            lt = pers.tile([2, NT, E], BF16, name=f"lhs{par}")
            rt = pers.tile([2, NT, E], BF16, name=f"rhs{par}")
            nc.gpsimd.memset(lt[:], 1.0)
            nc.gpsimd.memset(rt[:], 1.0)
            lhs_tiles[par] = lt
            rhs_tiles[par] = rt

        # ---------------- main-loop pools (SBUF) ----------------
        xTp = ctx.enter_context(tc.tile_pool(name="xTp", bufs=4))
        c8p = ctx.enter_context(tc.tile_pool(name="c8p", bufs=2))
        mskp = ctx.enter_context(tc.tile_pool(name="mskp", bufs=4))
        Ep = ctx.enter_context(tc.tile_pool(name="Ep", bufs=2))
        hsp = ctx.enter_context(tc.tile_pool(name="hsp", bufs=5))
        osbp = ctx.enter_context(tc.tile_pool(name="osbp", bufs=2))
        sml = ctx.enter_context(tc.tile_pool(name="sml", bufs=2))

        xt_tiles = {}
        msk_tiles = {}
        hs_tiles = {}
        osb_tiles = {}

        def emit_xt_dma(n):
            xt = xTp.tile([P, KC, E], BF16, tag="xT")
            nc.sync.dma_start(xt[:], xT[n].rearrange("(c p) j -> p c j", p=P))
            xt_tiles[n] = xt

        def emit_msk_dma(n):
            # SWDGE queue: keeps mask loads off the in-order SP DMA queue
            mk = mskp.tile([P, NT, EC, E], U8, tag="msk")
            nc.gpsimd.dma_start(mk[:], msk[n].rearrange("t (c p) j -> p t c j", p=P))
            msk_tiles[n] = mk

        def emit_h(n, ps_pool):
            """h = X @ W2 for graph n: 16 matmuls + 4 PSUM->SBUF copies."""
            hs = hsp.tile([P, EC, D], BF16, tag="hs")
            xt = xt_tiles[n]
            for ic in range(EC):
                ph = ps_pool.tile([P, D], F32, tag="m")
                for kc in range(KC):
                    nc.tensor.matmul(ph[:], xt[:, kc, ic * P:(ic + 1) * P],
                                     W2sb[:, kc, :],
                                     start=(kc == 0), stop=(kc == KC - 1))
                if ic % 2 == 0:
                    nc.vector.tensor_copy(hs[:, ic, :], ph[:])
                else:
                    nc.scalar.copy(hs[:, ic, :], ph[:])
            hs_tiles[n] = hs

        def emit_LR(n, ps_pool):
            """L/R score rows for graph n into the ping-pong operand tiles."""
            xt = xt_tiles[n]
            par = n % 2
            pLR = ps_pool.tile([P, E], F32, tag="m")
            for kc in range(KC):
                nc.tensor.matmul(pLR[0:2 * NT, :], U_all[:, kc, :, n],
                                 xt[:, kc, :],
                                 start=(kc == 0), stop=(kc == KC - 1))
            LR_sb = sml.tile([2 * NT, E], BF16, tag="lr")
            nc.scalar.copy(LR_sb[:], pLR[0:2 * NT, :])
            # SBUF->SBUF DMA gathers (engines cannot write partition base 1);
            # pLR rows are (L1,L2,L3,R1,R2,R3) so each gather is partition-
            # contiguous: 3 partitions -> 1 partition x 3 free chunks.
            nc.sync.dma_start(lhs_tiles[par][0:1, :, :], LR_sb[0:NT, :])
            nc.sync.dma_start(rhs_tiles[par][1:2, :, :], LR_sb[NT:2 * NT, :])

        # ---------------- prep phase: gates -> U (stage-major) ----------------
        with tc.tile_pool(name="prep", bufs=1) as prep, \
             tc.tile_pool(name="w2qp", bufs=2) as w2qp, \
             tc.tile_pool(name="pps", bufs=2, space="PSUM") as pps, \
             tc.tile_pool(name="pmh", bufs=2, space="PSUM") as pmh:
            qTsb = prep.tile([P, KC, NG], BF16)
            with nc.allow_non_contiguous_dma(reason="small qT load"):
                nc.sync.dma_start(qTsb[:], qT.rearrange("(c p) n -> p c n", p=P))
            aTsb = prep.tile([P, NT, 2 * KC, 1], BF16)
            with nc.allow_non_contiguous_dma(reason="small aT load"):
                nc.sync.dma_start(aTsb[:], arep.rearrange("t (c p) -> p t c", p=P)[:, :, :, None])
            W1sb = prep.tile([P, NT, KC, D2], BF16)
            WTsb = prep.tile([P, NT, KC, K], BF16)
            w2q_tiles = {}

            def emit_w2q_dma(t):
                # scalar queue: the rotating buffer wait must not block SP
                # DMAs; quarter-granularity so the g-stage races the load
                parts = []
                for hf in range(4):
                    w2qh = w2qp.tile([P, KC // 2, D2], BF16, tag=f"w2qh{hf}")
                    nc.scalar.dma_start(
                        w2qh[:],
                        W2q[t, hf * (D // 2):(hf + 1) * (D // 2)].rearrange(
                            "(c p) f -> p c f", p=P))
                    parts.append(w2qh)
                w2q_tiles[t] = parts

            nc.sync.dma_start(W1sb[:, 0], W1[0].rearrange("(c p) f -> p c f", p=P))
            emit_w2q_dma(0)
            nc.sync.dma_start(W1sb[:, 1], W1[1].rearrange("(c p) f -> p c f", p=P))
            emit_w2q_dma(1)
            nc.sync.dma_start(W1sb[:, 2], W1[2].rearrange("(c p) f -> p c f", p=P))
            emit_xt_dma(0)
            emit_msk_dma(0)
            emit_xt_dma(1)
            emit_xt_dma(2)
            emit_xt_dma(3)
            for t in range(NT):
                nc.sync.dma_start(WTsb[:, t], WT[t].rearrange("(c p) k -> p c k", p=P))

            rrT_sb = prep.tile([P, NT, 2 * KC, NG], BF16)
            gT_sb = prep.tile([P, NT, 2 * KC, NG], BF16)
            vT_sb = prep.tile([P, NT, 2 * KC, NG], BF16)

            emit_h(0, pmh)  # fills PE while gate DMAs stream

            # rrT = relu(W1_t^T @ q) directly in d2-major layout
            # (weights stationary: moving operand is the 8-column qT)
            for t in range(NT):
                rrps = pps.tile([P, 2 * KC, NG], F32, tag="p8")
                for oc in range(2 * KC):
                    for kc in range(KC):
                        nc.tensor.matmul(
                            rrps[:, oc, :],
                            W1sb[:, t, kc, oc * P:(oc + 1) * P],
                            qTsb[:, kc, :],
                            start=(kc == 0), stop=(kc == KC - 1))
                nc.scalar.activation(rrT_sb[:, t], rrps[:], AF.Relu)
            emit_h(1, pmh)
            # gT = sigmoid(W2q_t^T @ rrT), weights stationary
            for t in range(NT):
                halves = w2q_tiles[t]
                gps = pps.tile([P, 2 * KC, NG], F32, tag="p8")
                for oc in range(2 * KC):
                    for dc in range(2 * KC):
                        w2qh = halves[dc // 2]
                        nc.tensor.matmul(
                            gps[:, oc, :],
                            w2qh[:, dc % 2, oc * P:(oc + 1) * P],
                            rrT_sb[:, t, dc, :],
                            start=(dc == 0), stop=(dc == 2 * KC - 1))
                nc.scalar.activation(gT_sb[:, t], gps[:], AF.Sigmoid)
                if t == 0:
                    emit_w2q_dma(2)
            emit_h(2, pmh)
            # vT = gT * aT (broadcast over the n axis)
            for t in range(NT):
                nc.vector.tensor_tensor(
                    vT_sb[:, t], gT_sb[:, t],
                    aTsb[:, t].broadcast_to((P, 2 * KC, NG)), OP.mult)
            emit_msk_dma(1)
            emit_msk_dma(2)
            emit_msk_dma(3)
            # U = WT_t^T @ vT_half per k-chunk, weights stationary; lands
            # directly in U_all's k-major layout
            for kc in range(KC):
                ups = pps.tile([P, 2 * NT, NG], F32, tag="up")
                for s in range(2):
                    for t in range(NT):
                        for dc in range(KC):
                            nc.tensor.matmul(
                                ups[:, t + NT * s, :],
                                WTsb[:, t, dc, kc * P:(kc + 1) * P],
                                vT_sb[:, t, s * KC + dc, :],
                                start=(dc == 0), stop=(dc == KC - 1))
                nc.vector.tensor_copy(U_all[:, kc], ups[:])
            emit_LR(0, pmh)

        # ---------------- main per-graph pipeline ----------------
        ps_cand = ctx.enter_context(tc.tile_pool(name="ps_cand", bufs=3,
                                                 space="PSUM"))
        ps_misc = ctx.enter_context(tc.tile_pool(name="ps_misc", bufs=2,
                                                 space="PSUM"))

        def emit_escore_ic(n, ic, E_sb):
            """cand matmuls -> selects -> prelu for one i-chunk.

            1-wide with 3-deep A/B bank rotations so selects of adjacent
            i-chunks (and adjacent graphs) overlap instead of serializing
            on PSUM banks."""
            mk = msk_tiles[n]
            par = n % 2
            pa = ps_cand.tile([P, E], F32, tag="cA")
            nc.tensor.matmul(pa[:], lhs_tiles[par][:, 0, ic * P:(ic + 1) * P],
                             rhs_tiles[par][:, 0, :], start=True, stop=True)
            pb = ps_cand.tile([P, E], F32, tag="cB")
            nc.tensor.matmul(pb[:], lhs_tiles[par][:, 1, ic * P:(ic + 1) * P],
                             rhs_tiles[par][:, 1, :], start=True, stop=True)
            nc.vector.copy_predicated(pa[:], mk[:, 0, ic, :], pb[:])
            pb2 = ps_cand.tile([P, E], F32, tag="cB")
            nc.tensor.matmul(pb2[:], lhs_tiles[par][:, 2, ic * P:(ic + 1) * P],
                             rhs_tiles[par][:, 2, :], start=True, stop=True)
            nc.vector.copy_predicated(pa[:], mk[:, 1, ic, :], pb2[:])
            nc.vector.copy_predicated(pa[:], mk[:, 2, ic, :], negpl[:])
            if USE_PRELU:
                nc.scalar.activation(E_sb[:, ic, :], pa[:], AF.Prelu,
                                     alpha=LRELU_SLOPE)
            else:
                ab = sml.tile([P, E], F32, tag="ab")
                nc.scalar.activation(ab[:], pa[:], AF.Abs,
                                     scale=(1.0 - LRELU_SLOPE) / 2.0)
                nc.vector.scalar_tensor_tensor(
                    E_sb[:, ic, :], pa[:], (1.0 + LRELU_SLOPE) / 2.0,
                    ab[:], OP.mult, OP.add)

        def emit_soft(n, E_sb):
            """exp -> rowsum -> reciprocal -> normalized coefs, per i-chunk."""
            rs = sml.tile([P, EC], F32, tag="rs")
            rsr = sml.tile([P, EC], F32, tag="rsr")
            C8 = c8p.tile([P, EC, E], BF16, tag="C8")
            for ic in range(EC):
                # bias -2 keeps exp within fp16 range; cancels in softmax
                nc.scalar.activation(E_sb[:, ic, :], E_sb[:, ic, :], AF.Exp,
                                     bias=expbias[:, 0:1],
                                     accum_out=rs[:, ic:ic + 1])
                nc.vector.reciprocal(rsr[:, ic:ic + 1], rs[:, ic:ic + 1])
                nc.gpsimd.tensor_scalar(C8[:, ic, :], E_sb[:, ic, :],
                                        rsr[:, ic:ic + 1], None, OP.mult)
            return C8

        def emit_out(n, C8, jcs):
            """out = coefs^T @ h for graph n, j-chunks jcs."""
            hs = hs_tiles[n]
            if n in osb_tiles:
                osb = osb_tiles[n]
            else:
                osb = osbp.tile([P, EC, D], BF16, tag="osb")
                osb_tiles[n] = osb
            for jc in jcs:
                po = ps_misc.tile([P, D], F32, tag="m")
                for ic in range(EC):
                    nc.tensor.matmul(po[:], C8[:, ic, jc * P:(jc + 1) * P],
                                     hs[:, ic, :],
                                     start=(ic == 0), stop=(ic == EC - 1))
                nc.scalar.copy(osb[:, jc, :], po[:])
            if jcs[-1] == EC - 1:
                nc.sync.dma_start(out[n].rearrange("(c p) d -> p c d", p=P),
                                  osb[:])

        E_tiles = {}
        for n in range(NG):
            if n + 4 < NG:
                emit_xt_dma(n + 4)
            if n + 3 < NG:
                emit_h(n + 3, ps_misc)
            if n + 1 < NG:
                emit_LR(n + 1, ps_misc)
            C8 = emit_soft(n - 1, E_tiles.pop(n - 1)) if n >= 1 else None
            E_sb = Ep.tile([P, EC, E], BF16, tag="E")
            E_tiles[n] = E_sb
            emit_escore_ic(n, 0, E_sb)
            emit_escore_ic(n, 1, E_sb)
            if C8 is not None:
                emit_out(n - 1, C8, (0, 1))   # PE filler while selects run
            emit_escore_ic(n, 2, E_sb)
            if C8 is not None:
                emit_out(n - 1, C8, (2, 3))
            emit_escore_ic(n, 3, E_sb)
            if n + 4 < NG:
                emit_msk_dma(n + 4)
        # fused tail for the last graph: per-ic exp/recip/norm
        n = NG - 1
        E_sb = E_tiles.pop(n)
        rs = sml.tile([P, EC], F32, tag="rs")
        rsr = sml.tile([P, EC], F32, tag="rsr")
        C8 = c8p.tile([P, EC, E], BF16, tag="C8")
        for ic in range(EC):
            nc.scalar.activation(E_sb[:, ic, :], E_sb[:, ic, :], AF.Exp,
                                 bias=expbias[:, 0:1],
                                 accum_out=rs[:, ic:ic + 1])
            nc.vector.reciprocal(rsr[:, ic:ic + 1], rs[:, ic:ic + 1])
            nc.gpsimd.tensor_scalar(C8[:, ic, :], E_sb[:, ic, :],
                                    rsr[:, ic:ic + 1], None, OP.mult)
        emit_out(n, C8, (0, 1, 2, 3))
    return nc


_NC_CACHE = {}
TRACE = False
_LAST = {}


def _get_nc():
    if "nc" not in _NC_CACHE:
        nc = bacc.Bacc("TRN2", target_bir_lowering=False, debug=False)
        build(nc)
        nc.compile()
        _NC_CACHE["nc"] = nc
    return _NC_CACHE["nc"]


def kernel(input_state, adj, entity_mask, query_vec, W_type, a_type,
           qattn_W1, qattn_W2):
    from concourse import bass_utils
    nc = _get_nc()
    input_state = np.asarray(input_state, dtype=np.float32)
    adj = np.asarray(adj, dtype=np.int32)
    query_vec = np.asarray(query_vec, dtype=np.float32)
    W_type = np.asarray(W_type, dtype=np.float32)
    a_type = np.asarray(a_type, dtype=np.float32)
    qattn_W1 = np.asarray(qattn_W1, dtype=np.float32)
    qattn_W2 = np.asarray(qattn_W2, dtype=np.float32)

    xTf = np.ascontiguousarray(input_state.transpose(0, 2, 1))
    xT_all = xTf.astype(BF)
    msk_all = np.stack([(adj == 2), (adj == 3), (adj == 0)],
                       axis=1).astype(np.uint8)
    qT_all = np.ascontiguousarray(query_vec.T).astype(BF)
    W1_h = qattn_W1.astype(BF)
    W2q_h = qattn_W2.astype(BF)
    WT_h = np.ascontiguousarray(W_type.transpose(0, 2, 1)).astype(BF)
    W2_h = np.ascontiguousarray(W_type[2]).astype(BF)
    arep_h = np.ascontiguousarray(a_type).astype(BF)

    in_maps = []
    for c in range(N_CORES):
        sl = slice(c * NG, (c + 1) * NG)
        in_maps.append({
            "xT": xT_all[sl], "msk": msk_all[sl],
            "qT": np.ascontiguousarray(qT_all[:, sl]),
            "W1": W1_h, "W2q": W2q_h, "WT": WT_h, "W2": W2_h, "arep": arep_h,
        })
    res = bass_utils.run_bass_kernel_spmd(nc, in_maps, core_ids=list(range(N_CORES)),
                                          trace=TRACE, stitch_traces=TRACE)
    _LAST["exec_ns"] = res.exec_time_ns
    _LAST["mean_ns"] = res.mean_exec_time_ns
    out = np.concatenate([r["out"].astype(np.float32) for r in res.results],
                         axis=0)
    return out
